# revision 20
# baseline (speedup 1.0000x reference)
"""Trainium2 Bass kernel for FAMHA (spatial-reduction multi-head attention
with a 1x1 conv mixing attention heads before softmax).

Full (unsharded) inputs in, full output out. Internally: data-parallel over
batch across 8 NeuronCores (8 batches per core), dense per-core kernel built
with the Tile framework.

Math (per batch, fused/folded form):
  qT   = WqT.T @ queriesT                        (per o-chunk / q-chunk)
  x    = strided-subsample(queriesT) * sr_w + sr_b
  LN   : mu/rstd over channels via ones-matmul colsums, broadcast via K=1
         matmul, xn = (x - mu) * rstd   (ln_w/ln_b folded into Wk/Wv biases)
  kT   = (Wk*ln_w)T.T @ xn + bk'    v = xn.T @ (Wv*ln_w)T  (bv folded into
         the output-projection bias since sum(att)=1 after softmax)
  att'T[g] = sum_cc kmix[g].T @ qT   with kmix[g] = kT scaled by tw[g,h]/8
             (head-mix folded into K-side scaling; full 512 contraction)
  softmax (no max-subtraction; scores bounded by construction):
         e = exp(att'T); denominator via ones-column in v (row 64 of AV)
  AV   : av = v_aug.T @ e  -> [65, q]; attoutT = av[0:64] * (1/av[64]) bcast
  out  = attoutT.T @ WoT + (Wo @ bv' + bo)      (bias via K=1 ones matmul)
"""

import sys
import os

for _p in ("/opt/trn_rl_repo",):
    if _p not in sys.path and os.path.isdir(_p):
        sys.path.insert(0, _p)

import numpy as np
import concourse.bass as bass
import concourse.tile as tile
from concourse import mybir
from concourse.bass_utils import run_bass_kernel_spmd

F32 = mybir.dt.float32
F32R = mybir.dt.float32r
F16 = mybir.dt.float16

N_CORES = 8
B_TOTAL = 64
B = B_TOTAL // N_CORES  # batches per core
D = 512
H = 8
DK = 64
NQ = 784
NK = 196
HH = 28
QW = 392           # q free-dim chunk
NT, NW = 7, 112    # load/transpose tiles over nq
KS = ((0, 128), (128, 68))  # k-position splits (partition tiles of 196)
LN_EPS = 1e-5

Identity = mybir.ActivationFunctionType.Identity
Exp = mybir.ActivationFunctionType.Exp
Sqrt = mybir.ActivationFunctionType.Sqrt


def _split_excess_waits(nc):
    """This walrus build allows 1 sync wait per instruction (2 for
    EventSemaphore). Hoist excess waits emitted by the Tile scheduler onto
    same-engine InstNoOp carriers placed directly before the instruction."""
    n = 0
    for f in nc.m.functions:
        for bb in f.blocks:
            out = []
            dirty = False
            for ins in bb.instructions:
                si = ins.sync_info
                waits = list(si.on_wait) if si and si.on_wait else []
                limit = 2 if type(ins).__name__ == "InstEventSemaphore" else 1
                if len(waits) > limit:
                    for w in waits[:-limit]:
                        c = mybir.InstNoOp(name=f"{ins.name}-ws{n}", ins=[], outs=[])
                        c.engine = ins.engine
                        c.sync_info = mybir.SyncInfo(on_wait=[w], on_update=[])
                        out.append(c)
                        n += 1
                    ins.sync_info.on_wait = waits[-limit:]
                    dirty = True
                out.append(ins)
            if dirty:
                bb.instructions = out
    return n


def _bcast_mid(ap2d, n):
    """[P, F] AP -> [P, n, F] with a step-0 middle dim (free-dim broadcast)."""
    return bass.AP(
        tensor=ap2d.tensor,
        offset=ap2d.offset,
        ap=[list(ap2d.ap[0]), [0, n], list(ap2d.ap[1])],
    )


def _bcast_part_dram(ap_dram, n):
    """DRAM [1, F] AP -> [n, F] with a step-0 partition dim."""
    return bass.AP(
        tensor=ap_dram.tensor,
        offset=ap_dram.offset,
        ap=[[0, n]] + [list(x) for x in ap_dram.ap[1:]],
    )


class _Ctx:
    pass


def _alloc_consts(cx):
    nc, consts, MDT = cx.nc, cx.consts, cx.MDT
    cx.wq_sb = consts.tile([128, 4, D], MDT)
    cx.wk_sb = consts.tile([128, 4, D], MDT)
    cx.wv_sb = consts.tile([128, 4, D], MDT)
    cx.wo_sb = consts.tile([128, 4, D], MDT)
    nc.sync.dma_start(out=cx.wq_sb, in_=cx.wq_d.ap().rearrange("(cc p) o -> p cc o", p=128).bitcast(MDT))
    nc.sync.dma_start(out=cx.wk_sb, in_=cx.wk_d.ap().rearrange("(cc p) o -> p cc o", p=128).bitcast(MDT))
    nc.sync.dma_start(out=cx.wv_sb, in_=cx.wv_d.ap().rearrange("(cc p) o -> p cc o", p=128).bitcast(MDT))
    nc.sync.dma_start(out=cx.wo_sb, in_=cx.wo_d.ap().rearrange("(oc p) c -> p oc c", p=128).bitcast(MDT))
    cx.bq_sb = consts.tile([128, 4], F32)
    cx.bk_sb = consts.tile([128, 4], F32)
    cx.srw_sb = consts.tile([128, 4], F32)
    cx.srb_sb = consts.tile([128, 4], F32)
    cx.twc_sb = consts.tile([128, 32], F32)
    cx.ones_sb = consts.tile([128, 128], MDT)
    cx.ident_sb = consts.tile([128, 128], F32)
    cx.eps_sb = consts.tile([1, 1], F32)
    cx.cbv_sb = consts.tile([1, D], MDT)
    cx.ones_r = consts.tile([1, 128], F32R)
    cx.cbvh_sb = consts.tile([1, D], F16)
    cx.cbvl_sb = consts.tile([1, D], F16)
    cx.ones16_sb = consts.tile([1, 128], F16)
    nc.sync.dma_start(out=cx.bq_sb, in_=cx.bq_d[:, :])
    nc.sync.dma_start(out=cx.bk_sb, in_=cx.bk_d[:, :])
    nc.sync.dma_start(out=cx.srw_sb, in_=cx.srw_d[:, :])
    nc.sync.dma_start(out=cx.srb_sb, in_=cx.srb_d[:, :])
    nc.sync.dma_start(out=cx.twc_sb, in_=cx.twc_d[:, :])
    nc.sync.dma_start(out=cx.ones_sb, in_=cx.ones_d[:, :].bitcast(MDT))
    nc.sync.dma_start(out=cx.ident_sb, in_=cx.ident_d[:, :])
    nc.sync.dma_start(out=cx.eps_sb, in_=cx.eps_d[:, :])
    nc.sync.dma_start(out=cx.cbv_sb, in_=cx.cbv_d[:, :].bitcast(MDT))
    nc.sync.dma_start(out=cx.ones_r, in_=cx.ones_d[0:1, 0:128].bitcast(F32R))
    nc.sync.dma_start(out=cx.cbvh_sb, in_=cx.cbvh_d[:, :])
    nc.sync.dma_start(out=cx.cbvl_sb, in_=cx.cbvl_d[:, :])
    nc.sync.dma_start(out=cx.ones16_sb, in_=cx.ones16_d[:, :])


def _load_transpose(cx, b):
    """DMA queries[b] in naturally, PE-transpose to xT [c(4x128), nq]."""
    nc = cx.nc
    xTb = cx.p_xT.tile([128, 4, NQ], cx.MDT, tag="xT")
    for nt in range(NT):
        qn = cx.p_qnat.tile([128, D], F32, tag="qn")
        nc.sync.dma_start(out=qn[0:NW, :], in_=cx.q_in[b, nt * NW:(nt + 1) * NW, :])
        for ot in range(4):
            pt = cx.ps_a.tile([128, 512], F32, tag="ps_a")
            nc.tensor.transpose(
                pt[0:128, 0:NW],
                qn[0:NW, ot * 128:(ot + 1) * 128],
                cx.ident_sb[0:NW, 0:NW],
            )
            if (nt + ot) % 2 == 0:
                nc.vector.tensor_copy(out=xTb[:, ot, nt * NW:(nt + 1) * NW], in_=pt[0:128, 0:NW])
            else:
                nc.scalar.copy(xTb[:, ot, nt * NW:(nt + 1) * NW], pt[0:128, 0:NW])
    return xTb


def _sr_ln(cx, xT_b, xn, boff):
    """Spatial reduction + LayerNorm stats -> xn[:, :, boff:boff+NK]."""
    nc, MDT = cx.nc, cx.MDT
    xx2 = cx.p_xx2.tile([128, 4, 2 * NK], MDT, tag="xx2")
    for cc in range(4):
        xv = (
            xT_b[:, cc, :]
            .bitcast(F32)
            .rearrange("p (a b) -> p a b", b=HH)[:, 0:HH:2, 0:HH:2]
        )
        nc.scalar.activation(
            out=xx2[:, cc, 0:NK].rearrange("p (a b) -> p a b", b=14),
            in_=xv,
            func=Identity,
            bias=cx.srb_sb[:, cc:cc + 1],
            scale=cx.srw_sb[:, cc:cc + 1],
        )
    nc.vector.tensor_mul(xx2[:, :, NK:2 * NK], xx2[:, :, 0:NK], xx2[:, :, 0:NK])
    ps_s = cx.ps_a.tile([128, 512], F32, tag="ps_a")
    for cc in range(4):
        nc.tensor.matmul(
            ps_s[0:1, 0:2 * NK], cx.ones_sb[:, 0:1], xx2[:, cc, :],
            start=(cc == 0), stop=(cc == 3),
        )
    stat = cx.p_small.tile([1, 2 * NK], F32, tag="stat")
    nc.scalar.mul(stat, ps_s[0:1, 0:2 * NK], 1.0 / D)
    mu2 = cx.p_small.tile([1, NK], F32, tag="mu2")
    nc.vector.tensor_mul(mu2, stat[:, 0:NK], stat[:, 0:NK])
    nc.vector.tensor_sub(stat[:, NK:2 * NK], stat[:, NK:2 * NK], mu2)
    nc.scalar.activation(
        out=stat[:, NK:2 * NK], in_=stat[:, NK:2 * NK],
        func=Sqrt, bias=cx.eps_sb[0:1, 0:1], scale=1.0,
    )
    nc.vector.reciprocal(stat[:, NK:2 * NK], stat[:, NK:2 * NK])
    statr = cx.p_small.tile([1, 2 * NK], MDT, tag="statr")
    nc.scalar.copy(statr, stat)
    ps_b = cx.ps_a.tile([128, 512], F32, tag="ps_a")
    nc.tensor.matmul(ps_b[:, 0:2 * NK], cx.ones_sb[0:1, 0:128], statr, start=True, stop=True)
    nc.vector.tensor_sub(
        xn[:, :, boff:boff + NK], xx2[:, :, 0:NK], _bcast_mid(ps_b[:, 0:NK], 4)
    )
    nc.vector.tensor_mul(
        xn[:, :, boff:boff + NK], xn[:, :, boff:boff + NK],
        _bcast_mid(ps_b[:, NK:2 * NK], 4),
    )


def _k_proj(cx, xn):
    nc = cx.nc
    kT = cx.p_kT.tile([128, 4, 2 * NK], cx.MDT, tag="kT")
    for ot in range(4):
        ps_k = cx.ps_a.tile([128, 512], F32, tag="ps_a")
        for cc in range(4):
            nc.tensor.matmul(
                ps_k[:, 0:2 * NK],
                cx.wk_sb[:, cc, ot * 128:(ot + 1) * 128],
                xn[:, cc, :],
                start=(cc == 0), stop=(cc == 3),
            )
        nc.scalar.activation(
            out=kT[:, ot, :], in_=ps_k[:, 0:2 * NK],
            func=Identity, bias=cx.bk_sb[:, ot:ot + 1], scale=1.0,
        )
    return kT


def _v_proj(cx, xn, boff):
    nc = cx.nc
    v_tiles = []
    for (ko, kn) in KS:
        ps_v = cx.ps_a.tile([128, 512], F32, tag="ps_a")
        for cc in range(4):
            nc.tensor.matmul(
                ps_v[0:kn, 0:D],
                xn[:, cc, boff + ko:boff + ko + kn],
                cx.wv_sb[:, cc, :],
                start=(cc == 0), stop=(cc == 3),
            )
        vt = cx.p_v.tile([128, H, 65], cx.MDT, tag="vv")
        for h in range(H):
            if h % 2 == 0:
                nc.vector.tensor_copy(out=vt[0:kn, h, 0:64], in_=ps_v[0:kn, h * 64:(h + 1) * 64])
            else:
                nc.scalar.copy(vt[0:kn, h, 0:64], ps_v[0:kn, h * 64:(h + 1) * 64])
        nc.sync.dma_start(out=vt[0:kn, :, 64:65], in_=cx.ones_d[0:kn, 0:H].bitcast(cx.MDT))
        v_tiles.append(vt)
    return v_tiles


def _q_proj(cx, xT_b):
    nc = cx.nc
    if cx.qk_split:
        qh = cx.p_qTs.tile([128, 4, NQ], F16, tag="qTsh")
        ql = cx.p_qTs.tile([128, 4, NQ], F16, tag="qTsl")
    else:
        qh = cx.p_qTs.tile([128, 4, NQ], cx.MDT, tag="qTs")
        ql = None
    for ot in range(4):
        for qc in range(2):
            ps_q = cx.ps_a.tile([128, 512], F32, tag="ps_a")
            for cc in range(4):
                nc.tensor.matmul(
                    ps_q[:, 0:QW],
                    cx.wq_sb[:, cc, ot * 128:(ot + 1) * 128],
                    xT_b[:, cc, qc * QW:(qc + 1) * QW],
                    start=(cc == 0), stop=(cc == 3),
                )
            nc.scalar.activation(
                out=qh[:, ot, qc * QW:(qc + 1) * QW], in_=ps_q[:, 0:QW],
                func=Identity, bias=cx.bq_sb[:, ot:ot + 1], scale=1.0,
            )
            if ql is not None:
                # ql = (psum + bq) - qh  (exact fp16 residual)
                nc.vector.scalar_tensor_tensor(
                    out=ql[:, ot, qc * QW:(qc + 1) * QW],
                    in0=ps_q[:, 0:QW],
                    scalar=cx.bq_sb[:, ot:ot + 1],
                    in1=qh[:, ot, qc * QW:(qc + 1) * QW],
                    op0=mybir.AluOpType.add,
                    op1=mybir.AluOpType.subtract,
                )
    return (qh, ql)


def _attention(cx, kT, boff, qTs, v_tiles):
    nc, MDT = cx.nc, cx.MDT
    qh, ql = qTs
    ao = [cx.p_ao.tile([128, 4, QW], MDT, tag="ao", name=f"ao{i}") for i in range(2)]
    for g in range(H):
        if cx.qk_split:
            kmix = cx.p_kmix.tile([128, 4, NK], F32, tag="kmix")
        else:
            kmix = cx.p_kmix.tile([128, 4, NK], MDT, tag="kmix")
        for ot in range(4):
            tw_ap = cx.twc_sb[:, ot * 8 + g:ot * 8 + g + 1]
            kin = kT[:, ot, boff:boff + NK].bitcast(F32)
            if (g + ot) % 2 == 0:
                nc.vector.tensor_scalar_mul(out=kmix[:, ot, :], in0=kin, scalar1=tw_ap)
            else:
                nc.scalar.mul(kmix[:, ot, :], kin, tw_ap)
        if cx.qk_split:
            kh = cx.p_kmix.tile([128, 4, NK], F16, tag="kmixh")
            kl = cx.p_kmix.tile([128, 4, NK], F16, tag="kmixl")
            if g % 2 == 0:
                nc.scalar.copy(kh, kmix.bitcast(F32))
            else:
                nc.vector.tensor_copy(out=kh, in_=kmix.bitcast(F32))
            nc.vector.tensor_sub(kl, kmix.bitcast(F32), kh)
        for qc in range(2):
            att = cx.ps_att.tile([128, 2, 512], F32, tag="ps_att")
            for j, (ko, kn) in enumerate(KS):
                if cx.qk_split:
                    terms = (
                        (kh, qh), (kh, ql), (kl, qh),
                    )
                    for t, (kk, qq) in enumerate(terms):
                        for cc in range(4):
                            nc.tensor.matmul(
                                att[0:kn, j, 0:QW],
                                kk[:, cc, ko:ko + kn],
                                qq[:, cc, qc * QW:(qc + 1) * QW],
                                start=(t == 0 and cc == 0),
                                stop=(t == 2 and cc == 3),
                            )
                else:
                    for cc in range(4):
                        nc.tensor.matmul(
                            att[0:kn, j, 0:QW],
                            kmix[:, cc, ko:ko + kn],
                            qTs[0][:, cc, qc * QW:(qc + 1) * QW],
                            start=(cc == 0), stop=(cc == 3),
                        )
            e = cx.p_e.tile([128, 2 * QW], MDT, tag="esb")
            for j, (ko, kn) in enumerate(KS):
                nc.scalar.activation(
                    out=e[0:kn, j * QW:(j + 1) * QW],
                    in_=att[0:kn, j, 0:QW],
                    func=Exp,
                )
            av = cx.ps_av.tile([128, 512], F32, tag="ps_av")
            for j, (ko, kn) in enumerate(KS):
                nc.tensor.matmul(
                    av[0:65, 0:QW],
                    v_tiles[j][0:kn, g, :],
                    e[0:kn, j * QW:(j + 1) * QW],
                    start=(j == 0), stop=(j == 1),
                )
            recip = cx.p_recip.tile([1, QW], F32, tag="recip")
            nc.vector.reciprocal(recip, av[64:65, 0:QW])
            dden = cx.p_dram.tile([1, QW], F32, tag="dden")
            nc.sync.dma_start(out=dden, in_=recip)
            rb = cx.p_rb.tile([64, QW], F32, tag="rb")
            nc.sync.dma_start(out=rb, in_=_bcast_part_dram(dden[0:1, :], 64))
            ao_slice = ao[qc][(g % 2) * 64:(g % 2) * 64 + 64, g // 2, :]
            nc.vector.tensor_mul(ao_slice, av[0:64, 0:QW], rb)
    return ao


def _out_proj(cx, ao, b):
    nc = cx.nc
    for qc in range(2):
        for s in range(4):
            qt = qc * 4 + s
            ps_w = cx.ps_a.tile([128, 512], F32, tag="ps_a")
            nc.tensor.matmul(
                ps_w[0:98, 0:D], cx.ones16_sb[0:1, 0:98], cx.cbvh_sb,
                start=True, stop=False,
            )
            nc.tensor.matmul(
                ps_w[0:98, 0:D], cx.ones16_sb[0:1, 0:98], cx.cbvl_sb,
                start=False, stop=False,
            )
            for oc in range(4):
                nc.tensor.matmul(
                    ps_w[0:98, 0:D],
                    ao[qc][:, oc, s * 98:(s + 1) * 98],
                    cx.wo_sb[:, oc, :],
                    start=False, stop=(oc == 3),
                )
            ob = cx.p_out.tile([128, D], F32, tag="osb")
            if qt % 2 == 0:
                nc.vector.tensor_copy(out=ob[0:98, :], in_=ps_w[0:98, 0:D])
            else:
                nc.scalar.copy(ob[0:98, :], ps_w[0:98, 0:D])
            nc.sync.dma_start(out=cx.out_d[b, qt * 98:(qt + 1) * 98, :], in_=ob[0:98, :])


def _emit_body(cx, b_per_core, repeat):
    for _rep in range(repeat):
        for pair in range(b_per_core // 2):
            b0, b1 = 2 * pair, 2 * pair + 1
            xT = {b: _load_transpose(cx, b) for b in (b0, b1)}
            xn = cx.p_xn.tile([128, 4, 2 * NK], cx.MDT, tag="xn")
            for i, b in enumerate((b0, b1)):
                _sr_ln(cx, xT[b], xn, i * NK)
            kT = _k_proj(cx, xn)
            for i, b in enumerate((b0, b1)):
                boff = i * NK
                v_tiles = _v_proj(cx, xn, boff)
                qTs = _q_proj(cx, xT[b])
                ao = _attention(cx, kT, boff, qTs, v_tiles)
                _out_proj(cx, ao, b)


def build_nc(b_per_core=B, use_f32r=True, repeat=1, split_waits=True, qk_split=None):
    cx = _Ctx()
    cx.MDT = F32R if use_f32r else F32
    cx.qk_split = (not use_f32r) if qk_split is None else qk_split
    nc = bass.Bass("TRN2", target_bir_lowering=False, debug=False)
    cx.nc = nc

    cx.q_in = nc.declare_dram_parameter("q_in", [b_per_core, NQ, D], F32, isOutput=False)
    cx.wq_d = nc.declare_dram_parameter("wq", [D, D], F32, isOutput=False)   # [c, o]
    cx.wk_d = nc.declare_dram_parameter("wk", [D, D], F32, isOutput=False)   # [c, o]
    cx.wv_d = nc.declare_dram_parameter("wv", [D, D], F32, isOutput=False)   # [c, o]
    cx.wo_d = nc.declare_dram_parameter("wo", [D, D], F32, isOutput=False)   # [o, c]
    cx.bq_d = nc.declare_dram_parameter("bq_p", [128, 4], F32, isOutput=False)
    cx.bk_d = nc.declare_dram_parameter("bk_p", [128, 4], F32, isOutput=False)
    cx.srw_d = nc.declare_dram_parameter("srw_p", [128, 4], F32, isOutput=False)
    cx.srb_d = nc.declare_dram_parameter("srb_p", [128, 4], F32, isOutput=False)
    cx.twc_d = nc.declare_dram_parameter("twc_p", [128, 32], F32, isOutput=False)
    cx.ones_d = nc.declare_dram_parameter("ones_p", [128, 128], F32, isOutput=False)
    cx.ident_d = nc.declare_dram_parameter("ident_p", [128, 128], F32, isOutput=False)
    cx.eps_d = nc.declare_dram_parameter("eps_p", [1, 1], F32, isOutput=False)
    cx.cbv_d = nc.declare_dram_parameter("cbv_p", [1, D], F32, isOutput=False)
    cx.cbvh_d = nc.declare_dram_parameter("cbvh_p", [1, D], mybir.dt.float16, isOutput=False)
    cx.cbvl_d = nc.declare_dram_parameter("cbvl_p", [1, D], mybir.dt.float16, isOutput=False)
    cx.ones16_d = nc.declare_dram_parameter("ones16_p", [1, 128], mybir.dt.float16, isOutput=False)
    cx.out_d = nc.declare_dram_parameter("out", [b_per_core, NQ, D], F32, isOutput=True)

    with tile.TileContext(nc) as tc:
        with (
            tc.tile_pool(name="consts", bufs=1) as consts,
            tc.tile_pool(name="qnat", bufs=3) as p_qnat,
            tc.tile_pool(name="xT", bufs=2) as p_xT,
            tc.tile_pool(name="xx2", bufs=2) as p_xx2,
            tc.tile_pool(name="xn", bufs=2) as p_xn,
            tc.tile_pool(name="kT", bufs=2) as p_kT,
            tc.tile_pool(name="vv", bufs=4) as p_v,
            tc.tile_pool(name="qTs", bufs=2) as p_qTs,
            tc.tile_pool(name="kmix", bufs=2) as p_kmix,
            tc.tile_pool(name="esb", bufs=2) as p_e,
            tc.tile_pool(name="ao", bufs=3) as p_ao,
            tc.tile_pool(name="rb", bufs=3) as p_rb,
            tc.tile_pool(name="osb", bufs=4) as p_out,
            tc.tile_pool(name="small", bufs=2) as p_small,
            tc.tile_pool(name="recip", bufs=3) as p_recip,
            tc.tile_pool(name="ps_a", bufs=2, space="PSUM") as ps_a,
            tc.tile_pool(name="ps_att", bufs=2, space="PSUM") as ps_att,
            tc.tile_pool(name="ps_av", bufs=2, space="PSUM") as ps_av,
            tc.tile_pool(name="dram", bufs=4, space="DRAM") as p_dram,
        ):
            cx.consts = consts
            cx.p_qnat = p_qnat
            cx.p_xT = p_xT
            cx.p_xx2 = p_xx2
            cx.p_xn = p_xn
            cx.p_kT = p_kT
            cx.p_v = p_v
            cx.p_qTs = p_qTs
            cx.p_kmix = p_kmix
            cx.p_e = p_e
            cx.p_ao = p_ao
            cx.p_rb = p_rb
            cx.p_out = p_out
            cx.p_small = p_small
            cx.p_recip = p_recip
            cx.ps_a = ps_a
            cx.ps_att = ps_att
            cx.ps_av = ps_av
            cx.p_dram = p_dram
            _alloc_consts(cx)
            _emit_body(cx, b_per_core, repeat)

    if split_waits:
        _split_excess_waits(nc)
    return nc


def prep_consts(Wq, bq, Wk, bk, Wv, bv, Wo, bo, sr_w, sr_b, ln_w, ln_b, tw, tb):
    """Host-side constant folding. All fp32 numpy, all tiny. Note tb drops
    out of softmax entirely (constant along the key axis)."""
    Wq = np.asarray(Wq, np.float32); Wk = np.asarray(Wk, np.float32)
    Wv = np.asarray(Wv, np.float32); Wo = np.asarray(Wo, np.float32)
    ln_w = np.asarray(ln_w, np.float32); ln_b = np.asarray(ln_b, np.float32)
    tw = np.asarray(tw, np.float32)
    Wk_f = Wk * ln_w[None, :]
    Wv_f = Wv * ln_w[None, :]
    bk_f = np.asarray(bk, np.float32) + Wk @ ln_b
    bv_f = np.asarray(bv, np.float32) + Wv @ ln_b
    cbv = Wo @ bv_f + np.asarray(bo, np.float32)

    def col128(v):
        return np.ascontiguousarray(np.asarray(v, np.float32).reshape(4, 128).T)

    twc = np.zeros((128, 32), np.float32)
    for ot in range(4):
        for g in range(H):
            for p in range(128):
                twc[p, ot * 8 + g] = tw[g, (ot * 128 + p) // 64] / np.sqrt(DK)

    return {
        "wq": np.ascontiguousarray(Wq.T),
        "wk": np.ascontiguousarray(Wk_f.T),
        "wv": np.ascontiguousarray(Wv_f.T),
        "wo": np.ascontiguousarray(Wo.T),
        "bq_p": col128(bq),
        "bk_p": col128(bk_f),
        "srw_p": col128(sr_w),
        "srb_p": col128(sr_b),
        "twc_p": twc,
        "ones_p": np.ones((128, 128), np.float32),
        "ident_p": np.eye(128, dtype=np.float32),
        "eps_p": np.full((1, 1), LN_EPS, np.float32),
        "cbv_p": cbv.reshape(1, D).astype(np.float32),
        "cbvh_p": cbv.reshape(1, D).astype(np.float16),
        "cbvl_p": (cbv.reshape(1, D).astype(np.float32)
                   - cbv.reshape(1, D).astype(np.float16).astype(np.float32)).astype(np.float16),
        "ones16_p": np.ones((1, 128), np.float16),
    }


_NC_CACHE = {}


def _get_nc(b_per_core=B, use_f32r=False, repeat=1):
    key = (b_per_core, use_f32r, repeat)
    if key not in _NC_CACHE:
        _NC_CACHE[key] = build_nc(b_per_core, use_f32r, repeat)
    return _NC_CACHE[key]


def kernel(**inputs) -> np.ndarray:
    queries = np.ascontiguousarray(np.asarray(inputs["queries"], np.float32))
    consts = prep_consts(
        inputs["Wq"], inputs["bq"], inputs["Wk"], inputs["bk"],
        inputs["Wv"], inputs["bv"], inputs["Wo"], inputs["bo"],
        inputs["sr_w"], inputs["sr_b"], inputs["ln_w"], inputs["ln_b"],
        inputs["tw"], inputs["tb"],
    )
    nc = _get_nc(B, use_f32r=False, repeat=1)
    in_maps = []
    for c in range(N_CORES):
        m = dict(consts)
        m["q_in"] = np.ascontiguousarray(queries[c * B:(c + 1) * B])
        in_maps.append(m)
    res = run_bass_kernel_spmd(nc, in_maps, core_ids=list(range(N_CORES)))
    out = np.concatenate([res.results[c]["out"] for c in range(N_CORES)], axis=0)
    return out.astype(np.float32)
